# revision 4
# baseline (speedup 1.0000x reference)
"""EGRUBlock Trainium2 kernel.

Data-parallel across 8 NeuronCores: each core handles B_local=4 of the 32
sequences. Per core:
  Phase A: LayerNorm(x) in f32, cast to bf16, stage to DRAM scratch.
  Phase B: input projections az/ar/ah = xn @ W{z,r,h}.T + b (bf16 matmuls,
           f32 accum), staged to DRAM in a scan-friendly layout.
  Phase C: sequential GRU scan over T=2048 (bf16 matmuls vs the recurrent
           U matrices, f32 gate math / state), emitting h_t.
Host side: residual add (+x, exact f32) and batch re-assembly.
"""

import numpy as np
import ml_dtypes

import concourse.bass as bass
import concourse.mybir as mybir
import concourse.tile as tile
from concourse.bass import ds
from concourse.bass_utils import run_bass_kernel_spmd

BF16 = ml_dtypes.bfloat16

B, T, D, H = 32, 2048, 1024, 1024
EPS = 1e-5
N_CORES = 8
BL = B // N_CORES  # 4 sequences per core
KT = H // 128  # 8 k-tiles
ROWS = BL * T  # 8192 rows per core
RB = 512  # row-block for input GEMMs
N_RB = ROWS // RB  # 16
CH = 32  # scan chunk (steps per For_i iteration)

F32 = mybir.dt.float32
BF = mybir.dt.bfloat16


def _split_excess_waits(nc, max_waits=1):
    """walrus CoreV3 codegen in this env rejects >1 sync-wait per
    instruction; hoist extras onto preceding same-engine NoOps."""
    n = 0
    for fn in nc.m.functions:
        for blk in fn.blocks:
            insts = blk.instructions
            i = 0
            while i < len(insts):
                inst = insts[i]
                si = getattr(inst, "sync_info", None)
                if si is not None and si.on_wait and len(si.on_wait) > max_waits:
                    waits = list(si.on_wait)
                    extra, keep = waits[:-max_waits], waits[-max_waits:]
                    si.on_wait = keep
                    new_ops = []
                    for j in range(0, len(extra), max_waits):
                        chunk = extra[j : j + max_waits]
                        nop = mybir.InstNoOp(name=f"{inst.name}-ws{j}", ins=[], outs=[])
                        nop.engine = inst.engine
                        nop.sync_info = mybir.SyncInfo(on_wait=chunk, on_update=[])
                        new_ops.append(nop)
                        n += 1
                    insts[i:i] = new_ops
                    i += len(new_ops)
                i += 1
    return n


def build(scan_repeats=1):
    nc = bass.Bass("TRN2", target_bir_lowering=False, debug=False, num_devices=1)

    x_d = nc.dram_tensor("x", (BL, T, D), F32, kind="ExternalInput").ap()
    w_d = nc.dram_tensor("w_all", (3, D, H), BF, kind="ExternalInput").ap()
    u_d = nc.dram_tensor("u_all", (3, H, H), BF, kind="ExternalInput").ap()
    b_d = nc.dram_tensor("b_all", (3, KT, 128), F32, kind="ExternalInput").ap()
    gamma_d = nc.dram_tensor("gamma", (D,), F32, kind="ExternalInput").ap()
    beta_d = nc.dram_tensor("beta", (D,), F32, kind="ExternalInput").ap()
    y_d = nc.dram_tensor("y_dev", (128, KT, T * BL), F32, kind="ExternalOutput").ap()

    def bcast_ap(ap_1d, parts=128):
        return bass.AP(tensor=ap_1d.tensor, offset=ap_1d.offset,
                       ap=[[0, parts]] + list(ap_1d.ap))

    with tile.TileContext(nc) as tc:
        with (
            tc.tile_pool(name="singles", bufs=1) as singles,
            tc.tile_pool(name="dram", bufs=1, space="DRAM") as dram_pool,
        ):
            # ---- resident weights / constants ----
            w_sb = singles.tile([128, 3, KT, H], BF)
            nc.sync.dma_start(w_sb, w_d.rearrange("g (kt p) m -> p g kt m", p=128))
            u_sb = singles.tile([128, 3, KT, H], BF)
            nc.sync.dma_start(u_sb, u_d.rearrange("g (kt p) m -> p g kt m", p=128))
            bias_sb = singles.tile([128, 3, KT], F32)
            nc.sync.dma_start(bias_sb, b_d.rearrange("g m p -> p g m"))
            gamma_sb = singles.tile([128, D], F32)
            nc.gpsimd.dma_start(gamma_sb, bcast_ap(gamma_d))
            beta_sb = singles.tile([128, D], F32)
            nc.gpsimd.dma_start(beta_sb, bcast_ap(beta_d))
            eps_sb = singles.tile([128, 1], F32)
            nc.vector.memset(eps_sb, EPS)

            xn_blocks = [dram_pool.tile([RB, D], BF, name=f"xn_{i}") for i in range(N_RB)]
            # a_dram[g, mt, f, b, t]
            a_dram = dram_pool.tile([3, 128, KT * BL, T], BF, name="a_dram")

            x_flat = x_d.rearrange("b t d -> (b t) d")

            # ---------------- Phase A: LayerNorm ----------------
            with (
                tc.tile_pool(name="ln", bufs=3) as ln_pool,
                tc.tile_pool(name="ln_small", bufs=4) as ln_small,
            ):
                for it in range(ROWS // 128):
                    xt = ln_pool.tile([128, D], F32)
                    nc.sync.dma_start(xt, x_flat[ds(it * 128, 128)])
                    xg = xt.rearrange("p (s d) -> p s d", s=2)
                    stats = ln_small.tile([128, 2, nc.vector.BN_STATS_DIM], F32)
                    for s in range(2):
                        nc.vector.bn_stats(out=stats[:, s], in_=xg[:, s])
                    mv = ln_small.tile([128, nc.vector.BN_AGGR_DIM], F32)
                    nc.vector.bn_aggr(out=mv, in_=stats)
                    rstd = ln_small.tile([128, 1], F32)
                    nc.scalar.activation(out=rstd, in_=mv[:, 1:2],
                                         func=mybir.ActivationFunctionType.Sqrt,
                                         bias=eps_sb, scale=1.0, alpha=0.0)
                    nc.vector.reciprocal(out=rstd, in_=rstd)
                    nc.vector.tensor_scalar(out=xt, in0=xt,
                                            scalar1=mv[:, 0:1], scalar2=rstd,
                                            op0=mybir.AluOpType.subtract,
                                            op1=mybir.AluOpType.mult)
                    nc.vector.tensor_mul(out=xt, in0=xt, in1=gamma_sb)
                    xb = ln_pool.tile([128, D], BF, tag="xb")
                    nc.vector.tensor_add(out=xb, in0=xt, in1=beta_sb)
                    rb, loc = divmod(it * 128, RB)
                    nc.sync.dma_start(xn_blocks[rb][ds(loc, 128)], xb)

            # ---------------- Phase B: input GEMMs ----------------
            with (
                tc.tile_pool(name="gemm", bufs=3) as gemm_pool,
                tc.tile_pool(name="gemm_ps", bufs=4, space="PSUM") as gemm_ps,
            ):
                for rb in range(N_RB):
                    b_idx, tblk = divmod(rb, T // RB)
                    xnT = gemm_pool.tile([128, KT, RB], BF, tag="xnT")
                    nc.sync.dma_start_transpose(xnT, xn_blocks[rb][:])
                    for g in range(3):
                        for m in range(KT):
                            ps = gemm_ps.tile([128, RB], F32, tag="ps")
                            for kt in range(KT):
                                nc.tensor.matmul(
                                    ps, lhsT=w_sb[:, g, kt, ds(m * 128, 128)],
                                    rhs=xnT[:, kt], start=(kt == 0), stop=(kt == KT - 1))
                            asb = gemm_pool.tile([128, RB], BF, tag="asb")
                            nc.vector.tensor_scalar_add(
                                out=asb, in0=ps, scalar1=bias_sb[:, g, m : m + 1])
                            nc.sync.dma_start(
                                a_dram[g, :, m * BL + b_idx, ds(tblk * RB, RB)], asb)

            # ---------------- Phase C: GRU scan ----------------
            with (
                tc.tile_pool(name="state", bufs=1) as state,
                tc.tile_pool(name="scan", bufs=2) as scan_pool,
                tc.tile_pool(name="scan_sm", bufs=3) as scan_sm,
                tc.tile_pool(name="scan_ps", bufs=2, space="PSUM") as scan_ps,
            ):
                h_sb = state.tile([128, KT, BL], F32)
                hb_sb = state.tile([128, KT, BL], BF)
                nc.vector.memset(h_sb, 0.0)
                nc.vector.memset(hb_sb, 0.0)

                a_view = a_dram[:]

                ZG, RG, HG = 0, 1, 2

                def chunk_body(t0):
                    a_ch = []
                    for g in range(3):
                        ag = scan_pool.tile([128, KT * BL, CH], BF, tag=f"a{g}")
                        nc.sync.dma_start(ag, a_view[g, :, :, ds(t0, CH)])
                        a_ch.append(ag.rearrange("p (m b) t -> p m b t", b=BL))
                    y_ch = scan_pool.tile([128, KT, CH * BL], F32, tag="ych")
                    y_ch_v = y_ch.rearrange("p m (t b) -> p m t b", b=BL)

                    for tl in range(CH):
                        r_ps = scan_ps.tile([128, KT, BL], F32, tag="rps")
                        z_ps = scan_ps.tile([128, KT, BL], F32, tag="zps")
                        t_ps = scan_ps.tile([128, KT, BL], F32, tag="tps")
                        for m in range(KT):
                            for kt in range(KT):
                                nc.tensor.matmul(
                                    r_ps[:, m], lhsT=u_sb[:, RG, kt, ds(m * 128, 128)],
                                    rhs=hb_sb[:, kt], start=(kt == 0), stop=(kt == KT - 1))
                        r_sb = scan_sm.tile([128, KT, BL], F32, tag="rsb")
                        nc.vector.tensor_add(out=r_sb, in0=r_ps, in1=a_ch[RG][:, :, :, tl])
                        nc.scalar.activation(out=r_sb, in_=r_sb,
                                             func=mybir.ActivationFunctionType.Sigmoid)
                        rh_sb = scan_sm.tile([128, KT, BL], BF, tag="rhsb")
                        nc.vector.tensor_mul(out=rh_sb, in0=r_sb, in1=h_sb)

                        for m in range(KT):
                            for kt in range(KT):
                                nc.tensor.matmul(
                                    z_ps[:, m], lhsT=u_sb[:, ZG, kt, ds(m * 128, 128)],
                                    rhs=hb_sb[:, kt], start=(kt == 0), stop=(kt == KT - 1))
                        z_sb = scan_sm.tile([128, KT, BL], F32, tag="zsb")
                        nc.vector.tensor_add(out=z_sb, in0=z_ps, in1=a_ch[ZG][:, :, :, tl])
                        nc.scalar.activation(out=z_sb, in_=z_sb,
                                             func=mybir.ActivationFunctionType.Sigmoid)

                        for m in range(KT):
                            for kt in range(KT):
                                nc.tensor.matmul(
                                    t_ps[:, m], lhsT=u_sb[:, HG, kt, ds(m * 128, 128)],
                                    rhs=rh_sb[:, kt], start=(kt == 0), stop=(kt == KT - 1))
                        t_sb = scan_sm.tile([128, KT, BL], F32, tag="tsb")
                        nc.vector.tensor_add(out=t_sb, in0=t_ps, in1=a_ch[HG][:, :, :, tl])
                        nc.scalar.activation(out=t_sb, in_=t_sb,
                                             func=mybir.ActivationFunctionType.Tanh)

                        # h = h + z*(htilde - h)
                        nc.vector.tensor_sub(out=t_sb, in0=t_sb, in1=h_sb)
                        nc.vector.tensor_mul(out=t_sb, in0=t_sb, in1=z_sb)
                        nc.vector.tensor_add(out=h_sb, in0=h_sb, in1=t_sb)
                        nc.vector.tensor_copy(out=y_ch_v[:, :, tl], in_=h_sb)
                        nc.vector.tensor_copy(out=hb_sb, in_=h_sb)

                    nc.sync.dma_start(y_d[:, :, ds(t0 * BL, CH * BL)], y_ch)

                if scan_repeats == 1:
                    with tc.For_i(0, T, CH) as t0:
                        chunk_body(t0)
                else:
                    with tc.For_i(0, scan_repeats, 1):
                        with tc.For_i(0, T, CH) as t0:
                            chunk_body(t0)

    _split_excess_waits(nc)
    return nc


_nc_cache = {}


def _get_nc(scan_repeats=1):
    if scan_repeats not in _nc_cache:
        _nc_cache[scan_repeats] = build(scan_repeats)
    return _nc_cache[scan_repeats]


def make_in_maps(inputs):
    x = np.asarray(inputs["x"], np.float32)
    w_all = np.stack([np.asarray(inputs[k], np.float32).T for k in ("Wz", "Wr", "Wh")])
    u_all = np.stack([np.asarray(inputs[k], np.float32).T for k in ("Uz", "Ur", "Uh")])
    b_all = np.stack([np.asarray(inputs[k], np.float32) for k in ("bz", "br", "bh")])
    shared = {
        "w_all": w_all.astype(BF16),
        "u_all": u_all.astype(BF16),
        "b_all": b_all.reshape(3, KT, 128),
        "gamma": np.asarray(inputs["gamma"], np.float32),
        "beta": np.asarray(inputs["beta"], np.float32),
    }
    return [dict(shared, x=np.ascontiguousarray(x[c * BL : (c + 1) * BL]))
            for c in range(N_CORES)]


def assemble(results, x):
    ys = []
    for c in range(N_CORES):
        y_dev = results[c]["y_dev"].reshape(128, KT, T, BL)
        ys.append(y_dev.transpose(3, 2, 1, 0).reshape(BL, T, H))
    return np.concatenate(ys, axis=0) + np.asarray(x, np.float32)


def kernel(**inputs):
    nc = _get_nc(1)
    in_maps = make_in_maps(inputs)
    res = run_bass_kernel_spmd(nc, in_maps, core_ids=list(range(N_CORES)))
    return assemble(res.results, inputs["x"])
